# revision 7
# baseline (speedup 1.0000x reference)
"""Haar DWT (512x512, level 1) on 8 Trainium2 NeuronCores.

Input  x: [8, 64, 512, 512] f32 (the four Haar band matrices are fixed and
hardcoded into the kernel math). Output: (LL, LH, HL, HH), [8,64,256,256] f32.

Pure data parallel over batch: core i handles x[i]. Per core the separable
Haar transform is a 2x2 butterfly computed in two DVE stages:
  row stage : sums/difs of adjacent row pairs (full-width, contiguous)
  col stage : sums/difs of adjacent column pairs (stride-2 reads), written
              directly in the store layout (j, band, q)
and a final x0.5 on the scalar engine which also converts layout to f32.

DMA structure (the part that matters — the kernel is HBM-bound):
  * supertile = 4 images: one [128, 8192] f32 load (4 MiB, 32 KB
    contiguous per partition) on the sync HWDGE ring
  * ONE merged 4 MiB store per supertile into a [64, 256, 4, 256] output
    tensor (img, outrow, band, q): 32 KB contiguous per partition, scalar ring
  * mid tiles are bf16 to fit double-buffering of the 4 MiB pipeline in SBUF
4 MiB DMA granularity makes the read/write interleave phase-robust across
the cores sharing an HBM stack: measured ~358-360 us consistently, vs a
bimodal 326-408 us at 2 MiB granularity (~430 GB/s when cores align,
~350 GB/s when launch skew de-phases them).
"""

import numpy as np


def _ensure_concourse():
    try:
        import concourse.bass  # noqa: F401
    except ImportError:
        import sys

        for p in ("/opt/trn_rl_repo", "/root/.axon_site/_ro/trn_rl_repo"):
            if p not in sys.path:
                sys.path.append(p)
        import concourse.bass  # noqa: F401


N_CORES = 8
IMG = 512
N_IMAGES = 64
TAIL = 4  # trailing images processed as 1-image tiles (shorter drain)


def build_nc(barrier=False, io_bufs=2, mid_bufs=2, ws_bufs=2, ci_head=4):
    _ensure_concourse()
    from concourse import bacc, mybir
    from concourse.tile import TileContext

    f32 = mybir.dt.float32
    bf16 = mybir.dt.bfloat16

    nc = bacc.Bacc("TRN2", target_bir_lowering=False, debug=False)

    x = nc.dram_tensor("x", [N_IMAGES, IMG, IMG], f32, kind="ExternalInput")
    # [img, outrow, band, q]: partition (c g) stores rows 4g..4g+3 x 4 bands
    # x 256 q = one contiguous 16 KB run per partition
    o = nc.dram_tensor("o", [N_IMAGES, IMG // 2, 4, IMG // 2], f32, kind="ExternalOutput")

    if barrier:
        # all-core entry barrier: gate the load-issuing engine on the prelude
        # AllGather's semaphore. Emitted OUTSIDE the TileContext so its
        # scheduling sim (which doesn't model the prelude increment) doesn't
        # deadlock on the wait.
        nc._bir_kernel_barrier_sem_replica_groups.append(set(range(N_CORES)))
        nc.sync.wait_ge(nc._bir_kernel_barrier_sem, nc.bir_kernel_barrier_sem_inc)
    with TileContext(nc) as tc:
        with (
            tc.tile_pool(name="io", bufs=io_bufs) as io_pool,
            tc.tile_pool(name="mid", bufs=mid_bufs) as mid_pool,
            tc.tile_pool(name="ws", bufs=ws_bufs) as ws_pool,
        ):
            def emit(xv_s, ov_s, ci):
                jn = 2 * ci
                fx = 2048 * ci
                # f32 HWDGE load: 4 MiB on the sync ring. (bf16 SWDGE cast
                # loads are ~25 us faster when the cores stay phase-aligned but
                # reintroduce a degraded ~400 us mode on de-phased stacks; the
                # graded time is max-over-cores, so phase-robust f32 wins.)
                xt = io_pool.tile([128, fx], f32, tag="x")
                nc.sync.dma_start(out=xt[:], in_=xv_s)

                # row stage: input rows u = 2j + eo; sums -> M[:, :fx/2],
                # difs -> M[:, fx/2:]
                x4 = xt[:].rearrange("p (j eo w) -> p j eo w", j=jn, eo=2)
                M = mid_pool.tile([128, fx], bf16, tag="mid")
                M4 = M[:].rearrange("p (h j w) -> p h j w", h=2, j=jn)
                nc.vector.tensor_add(M4[:, 0], x4[:, :, 0, :], x4[:, :, 1, :])
                nc.vector.tensor_sub(M4[:, 1], x4[:, :, 0, :], x4[:, :, 1, :])

                # col stage: w = 2q + t, fused over sums|difs; write wr in the
                # store layout (j, band, q) with band = 2h + xsel
                wr = mid_pool.tile([128, fx], bf16, tag="wraw")
                ws = ws_pool.tile([128, fx], f32, tag="wsc")
                M5 = M[:].rearrange("p (h j q two) -> p h j q two", h=2, j=jn, two=2)
                wr5 = wr[:].rearrange("p (j h xsel q) -> p h xsel j q", j=jn, h=2, xsel=2)
                nc.vector.tensor_add(wr5[:, :, 0], M5[:, :, :, :, 0], M5[:, :, :, :, 1])
                nc.vector.tensor_sub(wr5[:, :, 1], M5[:, :, :, :, 0], M5[:, :, :, :, 1])

                nc.scalar.mul(ws[:], wr[:], 0.5)
                nc.scalar.dma_start(out=ov_s, in_=ws[:])

            # first HEAD images as 1-image tiles: the first store reaches the
            # ring ~30 us earlier, cutting the loads-only half-rate ramp
            HEAD = 4
            bulk_end = N_IMAGES - TAIL
            xvH = x[:HEAD].rearrange("(s c) (g u) w -> s (c g) (u w)", c=1, u=4)
            ovH = o[:HEAD].rearrange("(s c) (g j) band q -> s (c g) (j band q)", c=1, j=2)
            for s in range(HEAD):
                emit(xvH[s], ovH[s], 1)
            xv = x[HEAD:bulk_end].rearrange(
                "(s c) (g u) w -> s (c g) (u w)", c=ci_head, u=4 * ci_head
            )
            ov = o[HEAD:bulk_end].rearrange(
                "(s c) (g j) band q -> s (c g) (j band q)", c=ci_head, j=2 * ci_head
            )
            for s in range((bulk_end - HEAD) // ci_head):
                emit(xv[s], ov[s], ci_head)
            xvB = x[bulk_end:].rearrange("(s c) (g u) w -> s (c g) (u w)", c=1, u=4)
            ovB = o[bulk_end:].rearrange(
                "(s c) (g j) band q -> s (c g) (j band q)", c=1, j=2
            )
            for s in range(TAIL):
                emit(xvB[s], ovB[s], 1)

    nc.compile()
    return nc


_NC_CACHE = {}


def _get_nc(barrier=False):
    if barrier not in _NC_CACHE:
        _NC_CACHE[barrier] = build_nc(barrier=barrier)
    return _NC_CACHE[barrier]


def kernel(x, **_unused_matrices):
    """Full-input entry point: x [8, 64, 512, 512] f32 -> (LL, LH, HL, HH)."""
    _ensure_concourse()
    from concourse.bass_utils import run_bass_kernel_spmd

    x = np.ascontiguousarray(np.asarray(x, dtype=np.float32))
    assert x.shape == (N_CORES, N_IMAGES, IMG, IMG), x.shape

    in_maps = [{"x": x[i]} for i in range(N_CORES)]

    nc = _get_nc()
    try:
        res = run_bass_kernel_spmd(nc, in_maps, core_ids=list(range(N_CORES)))
    except ImportError:
        # BASS_TRACE forced but no NTFF hook available: run untraced.
        import os

        os.environ["BASS_NEVER_TRACE"] = "1"
        res = run_bass_kernel_spmd(nc, in_maps, core_ids=list(range(N_CORES)))
    r = res.results
    out = np.stack([r[i]["o"] for i in range(N_CORES)])  # [8, 64, 256, 4, 256]
    return tuple(np.ascontiguousarray(out[:, :, :, b, :]) for b in range(4))


# revision 10
# speedup vs baseline: 1.0200x; 1.0200x over previous
"""Haar DWT (512x512, level 1) on 8 Trainium2 NeuronCores.

Input  x: [8, 64, 512, 512] f32 (the four Haar band matrices are fixed and
hardcoded into the kernel math). Output: (LL, LH, HL, HH), [8,64,256,256] f32.

Pure data parallel over batch: core i handles x[i]. Per core the separable
Haar transform is a 2x2 butterfly computed in two DVE stages:
  row stage : sums/difs of adjacent row pairs (full-width, contiguous)
  col stage : sums/difs of adjacent column pairs (stride-2 reads), written
              directly in the store layout (j, band, q)
and a final x0.5 on the scalar engine which also converts layout to f32.

DMA structure (the part that matters — the kernel is HBM-bound):
  * supertile = 4 images: one [128, 8192] load (4 MiB of f32 HBM reads,
    cast to bf16 on the fly) on the SWDGE (gpsimd) queue
  * ONE merged 4 MiB store per supertile into a [64, 256, 4, 256] output
    tensor (img, outrow, band, q): 32 KB contiguous per partition, scalar ring
  * mid tiles are bf16 to fit double-buffering of the 4 MiB pipeline in SBUF
4 MiB DMA granularity makes the read/write interleave phase-robust across
the cores sharing an HBM stack: measured ~358-360 us consistently, vs a
bimodal 326-408 us at 2 MiB granularity (~430 GB/s when cores align,
~350 GB/s when launch skew de-phases them).
"""

import numpy as np


def _ensure_concourse():
    try:
        import concourse.bass  # noqa: F401
    except ImportError:
        import sys

        for p in ("/opt/trn_rl_repo", "/root/.axon_site/_ro/trn_rl_repo"):
            if p not in sys.path:
                sys.path.append(p)
        import concourse.bass  # noqa: F401


N_CORES = 8
IMG = 512
N_IMAGES = 64
TAIL = 4  # trailing images processed as 1-image tiles (shorter drain)


def build_nc(barrier=False, io_bufs=3, mid_bufs=2, ws_bufs=2, ci_head=4):
    _ensure_concourse()
    from concourse import bacc, mybir
    from concourse.tile import TileContext

    f32 = mybir.dt.float32
    bf16 = mybir.dt.bfloat16

    nc = bacc.Bacc("TRN2", target_bir_lowering=False, debug=False)

    x = nc.dram_tensor("x", [N_IMAGES, IMG, IMG], f32, kind="ExternalInput")
    # [img, outrow, band, q]: partition (c g) stores rows 4g..4g+3 x 4 bands
    # x 256 q = one contiguous 16 KB run per partition
    o = nc.dram_tensor("o", [N_IMAGES, IMG // 2, 4, IMG // 2], f32, kind="ExternalOutput")

    if barrier:
        # all-core entry barrier: gate the load-issuing engine on the prelude
        # AllGather's semaphore. Emitted OUTSIDE the TileContext so its
        # scheduling sim (which doesn't model the prelude increment) doesn't
        # deadlock on the wait.
        nc._bir_kernel_barrier_sem_replica_groups.append(set(range(N_CORES)))
        nc.sync.wait_ge(nc._bir_kernel_barrier_sem, nc.bir_kernel_barrier_sem_inc)
    with TileContext(nc) as tc:
        with (
            tc.tile_pool(name="io", bufs=io_bufs) as io_pool,
            tc.tile_pool(name="mid", bufs=mid_bufs) as mid_pool,
            tc.tile_pool(name="ws", bufs=ws_bufs) as ws_pool,
        ):
            def emit(xv_s, ov_s, ci, hwdge_ld=False):
                jn = 2 * ci
                fx = 2048 * ci
                if hwdge_ld:
                    # head tiles: f32 load on the (idle during ramp) sync ring
                    # at full HWDGE rate; row stage takes f32 in / bf16 out
                    xt = io_pool.tile([128, fx], f32, tag="x")
                    nc.sync.dma_start(out=xt[:], in_=xv_s)
                else:
                    # SWDGE cast load: HBM reads f32, SBUF receives bf16 —
                    # halves load-side fabric traffic and enables the DVE 2x
                    # (16-bit step-1) mode for the row stage
                    xt = io_pool.tile([128, fx], bf16, tag="x")
                    nc.gpsimd.dma_start(out=xt[:], in_=xv_s)

                # row stage: input rows u = 2j + eo; sums -> M[:, :fx/2],
                # difs -> M[:, fx/2:]
                x4 = xt[:].rearrange("p (j eo w) -> p j eo w", j=jn, eo=2)
                M = mid_pool.tile([128, fx], bf16, tag="mid")
                M4 = M[:].rearrange("p (h j w) -> p h j w", h=2, j=jn)
                nc.vector.tensor_add(M4[:, 0], x4[:, :, 0, :], x4[:, :, 1, :])
                nc.vector.tensor_sub(M4[:, 1], x4[:, :, 0, :], x4[:, :, 1, :])

                # col stage: w = 2q + t, fused over sums|difs; write wr in the
                # store layout (j, band, q) with band = 2h + xsel
                wr = mid_pool.tile([128, fx], bf16, tag="wraw")
                ws = ws_pool.tile([128, fx], f32, tag="wsc")
                M5 = M[:].rearrange("p (h j q two) -> p h j q two", h=2, j=jn, two=2)
                wr5 = wr[:].rearrange("p (j h xsel q) -> p h xsel j q", j=jn, h=2, xsel=2)
                nc.vector.tensor_add(wr5[:, :, 0], M5[:, :, :, :, 0], M5[:, :, :, :, 1])
                nc.vector.tensor_sub(wr5[:, :, 1], M5[:, :, :, :, 0], M5[:, :, :, :, 1])

                nc.scalar.mul(ws[:], wr[:], 0.5)
                nc.scalar.dma_start(out=ov_s, in_=ws[:])

            # first HEAD images as 1-image tiles: the first store reaches the
            # ring ~30 us earlier, cutting the loads-only half-rate ramp
            HEAD = 4
            bulk_end = N_IMAGES - TAIL
            xvH = x[:HEAD].rearrange("(s c) (g u) w -> s (c g) (u w)", c=1, u=4)
            ovH = o[:HEAD].rearrange("(s c) (g j) band q -> s (c g) (j band q)", c=1, j=2)
            for s in range(HEAD):
                emit(xvH[s], ovH[s], 1, hwdge_ld=True)
            xv = x[HEAD:bulk_end].rearrange(
                "(s c) (g u) w -> s (c g) (u w)", c=ci_head, u=4 * ci_head
            )
            ov = o[HEAD:bulk_end].rearrange(
                "(s c) (g j) band q -> s (c g) (j band q)", c=ci_head, j=2 * ci_head
            )
            for s in range((bulk_end - HEAD) // ci_head):
                emit(xv[s], ov[s], ci_head)
            xvB = x[bulk_end:].rearrange("(s c) (g u) w -> s (c g) (u w)", c=1, u=4)
            ovB = o[bulk_end:].rearrange(
                "(s c) (g j) band q -> s (c g) (j band q)", c=1, j=2
            )
            for s in range(TAIL):
                # sync ring is idle during the drain too: f32 HWDGE tail loads
                emit(xvB[s], ovB[s], 1, hwdge_ld=True)

    nc.compile()
    return nc


_NC_CACHE = {}


def _get_nc(barrier=False):
    if barrier not in _NC_CACHE:
        _NC_CACHE[barrier] = build_nc(barrier=barrier)
    return _NC_CACHE[barrier]


def kernel(x, **_unused_matrices):
    """Full-input entry point: x [8, 64, 512, 512] f32 -> (LL, LH, HL, HH)."""
    _ensure_concourse()
    from concourse.bass_utils import run_bass_kernel_spmd

    x = np.ascontiguousarray(np.asarray(x, dtype=np.float32))
    assert x.shape == (N_CORES, N_IMAGES, IMG, IMG), x.shape

    in_maps = [{"x": x[i]} for i in range(N_CORES)]

    nc = _get_nc()
    try:
        res = run_bass_kernel_spmd(nc, in_maps, core_ids=list(range(N_CORES)))
    except ImportError:
        # BASS_TRACE forced but no NTFF hook available: run untraced.
        import os

        os.environ["BASS_NEVER_TRACE"] = "1"
        res = run_bass_kernel_spmd(nc, in_maps, core_ids=list(range(N_CORES)))
    r = res.results
    out = np.stack([r[i]["o"] for i in range(N_CORES)])  # [8, 64, 256, 4, 256]
    return tuple(np.ascontiguousarray(out[:, :, :, b, :]) for b in range(4))
